# revision 15
# baseline (speedup 1.0000x reference)
"""Trainium2 Bass kernel for the DSAB block (nn_DSAB_block_61366492725647).

Contract: kernel(**inputs) takes the FULL unsharded inputs
(x: [8, 1024, 64, 64] f32 plus the 17 gate-weight tensors) and returns the
full output tuple (out_h, out_v), each [8, 1024, 64, 64] f32.

Strategy: data-parallel over batch B=8 across the 8 NeuronCores. Gate weights
are tiny and get host-packed into one [4, 32] tensor replicated to all cores.

This problem is memory-bound (per core: read x 16 MiB, write 32 MiB at f32).
The harness tolerance is 2e-2, so all bulk HBM traffic runs in bf16
(x in, out_h/out_v out: 25.2 MB/core total, ~0.6% worst-case rounding),
halving the DMA floor to ~70 us. Host converts f32<->bf16.

Per-core device kernel (x_b viewed [C=1024, S=4096] bf16, channels on
partitions). Measured engine rates that shaped the design: DVE tensor ops
~0.6 ns/elem in bf16 but tensor_reduce ~1.07 ns/elem regardless of dtype;
GPSIMD ~2 ns/elem; PE ~0.53 us per 512-col bf16 matmul; HWDGE rings move
~125/175/210 GB/s at 4/8/16 KiB lines. Hence: no big DVE reductions in the
streaming loop (PE reduces everything over channels), and all wide DMAs use
8 KiB lines.

  1. Stream x in as 8 tiles of [128, 4096] bf16 on both HWDGE rings
     (~2.9 us/tile cadence). Per tile:
       DVE:    g2 = w-pair fold (bf16), g8 = w-fold of g4 written straight
               into the stats tile, f2 = h-pair fold for the v path.
       GPSIMD: g4 = w-fold of g2.
       ACT:    diag / anti-diag gathers scaled by 64 (exact in bf16).
       PE:     one matmul reduces the stats tile [diag | anti | g8] over
               channels into psumS[1, 640]; four matmuls reduce f2 into
               psumV[1, 2048]. All bf16 single-pass with shared weights.
  2. Tail: R[1, 256] = [m_v | m_h | m_d | m_a] assembled on partition 0
     (DVE reduces psumV / psumS-g8 slices, ACT copies the gather sums),
     then one DMA scatters R to M4's four partitions (gate order v,h,d,a —
     matmul PSUM writes need base partition 0, so v's path stays there).
  3. Four LSK attention gates on [4, 64] tiles, conv taps as per-partition
     scalars (host-packed center row/col of the 5x5 / dilated 7x7).
  4. Gain maps G_h = attn_h * scale, G_v = attn_v * scale
     (scale = 1 + fusion_bias * diag projections) built as [64, 64] bf16
     partition-tiles from affine_select diagonal masks, flattened to a
     [1, 4096] row by DMA, then partition-broadcast by a single
     stride-0-source DMA per map ([128, 4096], one per ring) — the rings
     are idle at the phase boundary and this avoids GPSIMD broadcasts that
     stall concurrent DVE reads.
  5. out = x * G in [128, 4096] bf16 tiles on DVE (2.4 us each), written
     out as 1 MiB DMAs (8 KiB lines) alternating across both rings.
"""

from contextlib import ExitStack

import numpy as np

P = 128
C = 1024
HW = 64
S = HW * HW  # 4096
NT = C // P  # 8
B = 8

_CACHE = {}

# v first: the v-path PSUM accumulator must sit at base partition 0 (matmul
# PSUM writes only allow base 0/32/64), and with gate v on row 0 the gate
# means assemble contiguously on partition 0 as [m_v | m_h | m_d | m_a].
_GATE_ORDER = ("v", "h", "d", "a")


def _pack_gate_params(inputs):
    """Pack per-gate params into [4, 32] f32, one gate per row (v, h, d, a).

    cols 0:5   5-tap conv weights (center column of the 5x5 for the h gate,
               which convolves along H; center row for v/d/a)
    cols 5:12  7-tap conv weights (same center rule, dilation 3)
    col 12     ws[0,0]*0.5 (avg-branch weight, attn ch0; halved because the
               kernel feeds u1+u2 instead of (u1+u2)/2)
    col 13     ws[0,1] (max-branch weight, ch0)
    col 14     bs[0]
    col 15     ws[1,0]*0.5
    col 16     ws[1,1]
    col 17     bs[1]
    col 18     fusion_bias
    """
    gp = np.zeros((4, 32), np.float32)
    fb = float(np.asarray(inputs["fusion_bias"]).reshape(-1)[0])
    for g, n in enumerate(_GATE_ORDER):
        w0 = np.asarray(inputs[f"w{n}0"], np.float32)[0, 0]
        w1 = np.asarray(inputs[f"w{n}1"], np.float32)[0, 0]
        ws = np.asarray(inputs[f"w{n}s"], np.float32)[:, :, 0, 0]
        bs = np.asarray(inputs[f"b{n}s"], np.float32)
        along_h = n == "h"
        gp[g, 0:5] = w0[:, 2] if along_h else w0[2, :]
        gp[g, 5:12] = w1[:, 3] if along_h else w1[3, :]
        gp[g, 12] = ws[0, 0] * 0.5
        gp[g, 13] = ws[0, 1]
        gp[g, 14] = bs[0]
        gp[g, 15] = ws[1, 0] * 0.5
        gp[g, 16] = ws[1, 1]
        gp[g, 17] = bs[1]
        gp[g, 18] = fb
    return gp


def _emit(tc, outs, ins):
    import concourse.bass as bass
    import concourse.mybir as mybir

    F32 = mybir.dt.float32
    BF16 = mybir.dt.bfloat16
    AF = mybir.ActivationFunctionType
    OP = mybir.AluOpType

    nc = tc.nc
    x, gp = ins
    oh, ov = outs

    with ExitStack() as ctx:
        const = ctx.enter_context(tc.tile_pool(name="const", bufs=1))
        xpool = ctx.enter_context(tc.tile_pool(name="xp", bufs=1))
        small = ctx.enter_context(tc.tile_pool(name="small", bufs=1))
        gmaps = ctx.enter_context(tc.tile_pool(name="gmaps", bufs=1))
        res = ctx.enter_context(tc.tile_pool(name="res", bufs=4))
        g2p = ctx.enter_context(tc.tile_pool(name="g2p", bufs=2))
        g4p = ctx.enter_context(tc.tile_pool(name="g4p", bufs=2))
        f2p = ctx.enter_context(tc.tile_pool(name="f2p", bufs=2))
        stpool = ctx.enter_context(tc.tile_pool(name="stp", bufs=2))
        psum = ctx.enter_context(
            tc.tile_pool(name="ps", bufs=1, space=bass.MemorySpace.PSUM)
        )

        # ---- params / constants (emitted first so they schedule early) ----
        gpt = const.tile([4, 32], F32)
        nc.sync.dma_start(gpt[:], gp[:])
        ones1b = const.tile([128, 1], BF16)
        nc.vector.memset(ones1b[:], 1.0 / 65536.0)
        # binary diagonal / anti-diagonal masks, built on idle GPSIMD time
        ones64 = const.tile([64, 64], F32)
        nc.vector.memset(ones64[:], 1.0)
        mskD = const.tile([64, 64], F32)
        mskA = const.tile([64, 64], F32)
        nc.gpsimd.affine_select(
            mskD[:], ones64[:], [[1, 64]], OP.is_equal, 0.0,
            base=0, channel_multiplier=-1,
        )
        nc.gpsimd.affine_select(
            mskA[:], ones64[:], [[1, 64]], OP.is_equal, 0.0,
            base=-63, channel_multiplier=1,
        )

        # PSUM accumulators, both on partition 0.
        psumS = psum.tile([1, 640], F32)  # [diag*64 | anti*64 | g8 (64x8)]
        psumV = psum.tile([1, 2048], F32)  # f2 channel sums (32 h2 x 64 w)

        # force the Sigmoid ACT table to load during the idle in-phase
        # rather than on the gate critical path
        sigwarm = const.tile([1, 1], F32)
        nc.scalar.activation(sigwarm[:], gpt[0:1, 0:1], AF.Sigmoid)

        # ---- stream x in; per-tile work spread over DVE/ACT/GPS/PE ----
        xt = []
        for i in range(NT):
            t = xpool.tile([P, S], BF16, tag=f"x{i}", name=f"xt{i}")
            xt.append(t)
            eng = nc.sync if i % 2 == 0 else nc.scalar
            eng.dma_start(t[:], x[i * P : (i + 1) * P, :])
            x3 = t[:].rearrange("p (h w) -> p h w", h=HW)
            st = stpool.tile([P, 640], BF16, tag="st", name=f"st{i}")
            # diag / anti-diag gathers, pre-scaled by 64 (ACT; exact in bf16)
            nc.scalar.mul(st[:, 0:64], t[:, 0 : S : HW + 1], 64.0)
            nc.scalar.mul(st[:, 64:128], t[:, HW - 1 : S - HW + 1 : HW - 1], 64.0)
            # h path: fold w by 8 (DVE pair-fold, GPSIMD quad-fold, DVE final
            # fold written straight into the stats tile)
            g2 = g2p.tile([P, 2048], BF16, tag="g2", name=f"g2_{i}")
            g23 = g2[:].rearrange("p (h w) -> p h w", h=HW)
            nc.vector.tensor_tensor(
                g23, x3[:, :, 0:64:2], x3[:, :, 1:64:2], OP.add
            )
            g4 = g4p.tile([P, 1024], BF16, tag="g4", name=f"g4_{i}")
            g43 = g4[:].rearrange("p (h w) -> p h w", h=HW)
            nc.gpsimd.tensor_tensor(
                g43, g23[:, :, 0:32:2], g23[:, :, 1:32:2], OP.add
            )
            st8 = st[:, 128:640].rearrange("p (h w) -> p h w", h=HW)
            nc.vector.tensor_tensor(
                st8, g43[:, :, 0:16:2], g43[:, :, 1:16:2], OP.add
            )
            # v path: fold adjacent h rows (DVE), PE reduces over channels
            f2 = f2p.tile([P, 2048], BF16, tag="f2", name=f"f2_{i}")
            f23 = f2[:].rearrange("p (h w) -> p h w", h=32)
            nc.vector.tensor_tensor(f23, x3[:, 0:64:2, :], x3[:, 1:64:2, :], OP.add)
            # channel reductions on PE (bf16 single pass, shared weights;
            # matmul PSUM output is capped at 512 f32 cols per instruction)
            nc.tensor.matmul(
                psumS[0:1, 0:512], ones1b[:], st[:, 0:512],
                start=(i == 0), stop=(i == NT - 1),
            )
            nc.tensor.matmul(
                psumS[0:1, 512:640], ones1b[:], st[:, 512:640],
                start=(i == 0), stop=(i == NT - 1),
            )
            for j in range(4):
                sl = slice(j * 512, (j + 1) * 512)
                nc.tensor.matmul(
                    psumV[0:1, sl],
                    ones1b[:],
                    f2[:, sl],
                    start=(i == 0),
                    stop=(i == NT - 1),
                )

        # ---- tail: R = [m_v | m_h | m_d | m_a] on partition 0, one DMA
        # scatter into M4 [4, 64] (row g = gate g mean) ----
        M4 = small.tile([4, 64], F32)
        R = small.tile([1, 256], F32)
        nc.scalar.mul(R[0:1, 128:256], psumS[0:1, 0:128], 1.0)
        pv3 = psumV[0:1, :].rearrange("p (g w) -> p w g", g=32)
        nc.vector.reduce_sum(R[0:1, 0:64], pv3, axis=mybir.AxisListType.X)
        ph3 = psumS[0:1, 128:640].rearrange("p (h w) -> p h w", h=HW)
        nc.vector.reduce_sum(R[0:1, 64:128], ph3, axis=mybir.AxisListType.X)
        nc.sync.dma_start(M4[:], R[:])

        # ---- four gates on [4, 64]; row g = gate g ----
        def conv1d(dst, src, tap_base, ntaps, dil):
            c = ntaps // 2
            nc.vector.tensor_scalar(
                dst, src, gpt[:, tap_base + c : tap_base + c + 1], None, OP.mult
            )
            for k in range(ntaps):
                if k == c:
                    continue
                off = dil * (k - c)
                a0, b0 = max(0, -off), min(HW, HW - off)
                nc.vector.scalar_tensor_tensor(
                    dst[:, a0:b0],
                    src[:, a0 + off : b0 + off],
                    gpt[:, tap_base + k : tap_base + k + 1],
                    dst[:, a0:b0],
                    OP.mult,
                    OP.add,
                )

        u1 = small.tile([4, 64], F32)
        u2 = small.tile([4, 64], F32)
        conv1d(u1[:], M4[:], 0, 5, 1)
        conv1d(u2[:], u1[:], 5, 7, 3)

        sm = small.tile([4, 64], F32)  # u1+u2; the 0.5 lives in gp cols 12/15
        mx = small.tile([4, 64], F32)
        nc.vector.tensor_add(sm[:], u1[:], u2[:])
        nc.vector.tensor_tensor(mx[:], u1[:], u2[:], OP.max)
        z0 = small.tile([4, 64], F32)
        z1 = small.tile([4, 64], F32)
        nc.vector.tensor_scalar(z0[:], sm[:], gpt[:, 12:13], None, OP.mult)
        nc.vector.scalar_tensor_tensor(
            z0[:], mx[:], gpt[:, 13:14], z0[:], OP.mult, OP.add
        )
        nc.vector.tensor_scalar(z1[:], sm[:], gpt[:, 15:16], None, OP.mult)
        nc.vector.scalar_tensor_tensor(
            z1[:], mx[:], gpt[:, 16:17], z1[:], OP.mult, OP.add
        )
        at0 = small.tile([4, 64], F32)
        at1 = small.tile([4, 64], F32)
        nc.scalar.activation(at0[:], z0[:], AF.Sigmoid, bias=gpt[:, 14:15])
        nc.scalar.activation(at1[:], z1[:], AF.Sigmoid, bias=gpt[:, 17:18])
        nc.vector.tensor_mul(at0[:], u1[:], at0[:])
        nc.vector.tensor_mul(at1[:], u2[:], at1[:])
        nc.vector.tensor_add(at0[:], at0[:], at1[:])
        attn = small.tile([4, 64], F32)
        nc.scalar.activation(attn[:], at0[:], AF.Sigmoid)
        attnfb = small.tile([4, 64], F32)  # attn * fusion_bias (rows 2,3 used)
        nc.vector.tensor_scalar(attnfb[:], attn[:], gpt[:, 18:19], None, OP.mult)

        # ---- gain maps as [64, 64] bf16 partition-tiles (partition = h) ----
        ah_col = small.tile([64, 1], F32)
        fbd_col = small.tile([64, 1], F32)
        fba_col = small.tile([64, 1], F32)
        av = small.tile([1, 64], F32)
        avr = small.tile([64, 64], F32)
        nc.sync.dma_start(ah_col[:], attn[1:2, :])
        nc.sync.dma_start(fbd_col[:], attnfb[2:3, :])
        nc.scalar.dma_start(fba_col[:], attnfb[3:4, :])
        nc.scalar.dma_start(av[:], attn[0:1, :])
        nc.gpsimd.partition_broadcast(avr[:], av[:])

        # sum2d = fb*attn_d on diag + fb*attn_a on anti-diag (via 0/1 masks)
        sum2d = small.tile([64, 64], F32)
        nc.vector.tensor_scalar(sum2d[:], mskD[:], fbd_col[:], None, OP.mult)
        nc.vector.scalar_tensor_tensor(
            sum2d[:], mskA[:], fba_col[:], sum2d[:], OP.mult, OP.add
        )
        gh2d = small.tile([64, 64], BF16)
        gv2d = small.tile([64, 64], BF16)
        nc.vector.tensor_scalar(gh2d[:], sum2d[:], 1.0, ah_col[:], OP.add, OP.mult)
        nc.vector.scalar_tensor_tensor(
            gv2d[:], sum2d[:], 1.0, avr[:], OP.add, OP.mult
        )

        # flatten each map to a [1, 4096] row, then partition-broadcast with
        # a single stride-0-source DMA per map, one per ring
        ghrow = small.tile([1, S], BF16)
        gvrow = small.tile([1, S], BF16)
        nc.sync.dma_start(ghrow[:], gh2d[:])
        nc.scalar.dma_start(gvrow[:], gv2d[:])
        G_h = gmaps.tile([P, S], BF16)
        G_v = gmaps.tile([P, S], BF16)
        bh = ghrow[0:1, :].rearrange("p (o w) -> p o w", o=1).broadcast_to([1, P, S])
        bv = gvrow[0:1, :].rearrange("p (o w) -> p o w", o=1).broadcast_to([1, P, S])
        nc.sync.dma_start(G_h[:], bh)
        nc.scalar.dma_start(G_v[:], bv)

        # ---- out phase: out = x * G in [128, 4096] bf16 tiles (DVE),
        # written out as 1 MiB DMAs with 8 KiB lines ----
        for i in range(NT):
            osl = slice(i * P, (i + 1) * P)
            rh = res.tile([P, S], BF16, tag="res", name=f"rh{i}")
            nc.vector.tensor_mul(rh[:], xt[i][:], G_h[:])
            nc.sync.dma_start(oh[osl, :], rh[:])
            rv = res.tile([P, S], BF16, tag="res", name=f"rv{i}")
            nc.vector.tensor_mul(rv[:], xt[i][:], G_v[:])
            nc.scalar.dma_start(ov[osl, :], rv[:])


def _build_device_kernel():
    import concourse.bacc as bacc
    import concourse.mybir as mybir
    import concourse.tile as tile

    BF16 = mybir.dt.bfloat16
    F32 = mybir.dt.float32
    nc = bacc.Bacc("TRN2", target_bir_lowering=False, debug=False)
    x = nc.dram_tensor("x", [C, S], BF16, kind="ExternalInput").ap()
    gp = nc.dram_tensor("gp", [4, 32], F32, kind="ExternalInput").ap()
    oh = nc.dram_tensor("out_h", [C, S], BF16, kind="ExternalOutput").ap()
    ov = nc.dram_tensor("out_v", [C, S], BF16, kind="ExternalOutput").ap()

    with tile.TileContext(nc) as tc:
        _emit(tc, [oh, ov], [x, gp])

    nc.compile()
    return nc


def _get_nc():
    if "nc" not in _CACHE:
        _CACHE["nc"] = _build_device_kernel()
    return _CACHE["nc"]


def _run(inputs, **spmd_kwargs):
    """Shard, execute on 8 cores, gather. Returns (out_h, out_v, results)."""
    import ml_dtypes

    from concourse.bass_utils import run_bass_kernel_spmd

    nc = _get_nc()
    x = np.asarray(inputs["x"], dtype=np.float32)
    assert x.shape == (B, C, HW, HW), x.shape
    xb = np.ascontiguousarray(x.reshape(B, C, S).astype(ml_dtypes.bfloat16))
    gp = _pack_gate_params(inputs)
    in_maps = [{"x": xb[b], "gp": gp} for b in range(B)]
    r = run_bass_kernel_spmd(nc, in_maps, core_ids=list(range(B)), **spmd_kwargs)
    oh = np.stack([r.results[b]["out_h"] for b in range(B)])
    ov = np.stack([r.results[b]["out_v"] for b in range(B)])
    oh = oh.astype(np.float32).reshape(B, C, HW, HW)
    ov = ov.astype(np.float32).reshape(B, C, HW, HW)
    return oh, ov, r


def kernel(**inputs):
    oh, ov, _ = _run(inputs)
    return oh, ov


# revision 20
# speedup vs baseline: 1.5413x; 1.5413x over previous
"""Trainium2 Bass kernel for the DSAB block (nn_DSAB_block_61366492725647).

Contract: kernel(**inputs) takes the FULL unsharded inputs
(x: [8, 1024, 64, 64] f32 plus the 17 gate-weight tensors) and returns the
full output tuple (out_h, out_v), each [8, 1024, 64, 64] f32.

Strategy: data-parallel over batch B=8 across the 8 NeuronCores. Gate weights
are tiny and get host-packed into one [4, 32] tensor replicated to all cores.

This problem is memory-bound (per core: read x 16 MiB, write 32 MiB at f32).
The harness tolerance is 2e-2, so all bulk HBM traffic runs in bf16
(x in, out_h/out_v out: 25.2 MB/core total, ~0.6% worst-case rounding),
halving the DMA floor to ~70 us. Host converts f32<->bf16.

Per-core device kernel (x_b viewed [C=1024, S=4096] bf16, channels on
partitions). Measured engine rates that shaped the design: DVE tensor ops
~0.6 ns/elem in bf16 but tensor_reduce ~1.07 ns/elem regardless of dtype;
GPSIMD ~2 ns/elem; PE ~0.53 us per 512-col bf16 matmul; HWDGE rings move
~125/175/210 GB/s at 4/8/16 KiB lines. Hence: no big DVE reductions in the
streaming loop (PE reduces everything over channels), and all wide DMAs use
8 KiB lines.

  1. Stream x in as 8 tiles of [128, 4096] bf16 on both HWDGE rings
     (~2.9 us/tile cadence). Per tile:
       DVE:    g2 = w-pair fold (bf16), g8 = w-fold of g4 written straight
               into the stats tile, f2 = h-pair fold for the v path.
       GPSIMD: g4 = w-fold of g2.
       ACT:    diag / anti-diag gathers scaled by 64 (exact in bf16).
       PE:     one matmul reduces the stats tile [diag | anti | g8] over
               channels into psumS[1, 640]; four matmuls reduce f2 into
               psumV[1, 2048]. All bf16 single-pass with shared weights.
  2. Tail: R[1, 256] = [m_v | m_h | m_d | m_a] assembled on partition 0
     (DVE reduces psumV / psumS-g8 slices, ACT copies the gather sums),
     then one DMA scatters R to M4's four partitions (gate order v,h,d,a —
     matmul PSUM writes need base partition 0, so v's path stays there).
  3. Four LSK attention gates on [4, 64] tiles, conv taps as per-partition
     scalars (host-packed center row/col of the 5x5 / dilated 7x7).
  4. Gain maps G_h = attn_h * scale, G_v = attn_v * scale
     (scale = 1 + fusion_bias * diag projections) built as [64, 64] bf16
     partition-tiles from affine_select diagonal masks, flattened to a
     [1, 4096] row by DMA, then partition-broadcast by a single
     stride-0-source DMA per map ([128, 4096], one per ring) — the rings
     are idle at the phase boundary and this avoids GPSIMD broadcasts that
     stall concurrent DVE reads.
  5. out = x * G in [128, 4096] bf16 tiles on DVE (2.4 us each), written
     out as 1 MiB DMAs (8 KiB lines) alternating across both rings.
"""

from contextlib import ExitStack

import numpy as np

P = 128
C = 1024
HW = 64
S = HW * HW  # 4096
NT = C // P  # 8
B = 8

_CACHE = {}

# v first: the v-path PSUM accumulator must sit at base partition 0 (matmul
# PSUM writes only allow base 0/32/64), and with gate v on row 0 the gate
# means assemble contiguously on partition 0 as [m_v | m_h | m_d | m_a].
_GATE_ORDER = ("v", "h", "d", "a")


def _pack_gate_params(inputs):
    """Pack per-gate params into [4, 32] f32, one gate per row (v, h, d, a).

    cols 0:5   5-tap conv weights (center column of the 5x5 for the h gate,
               which convolves along H; center row for v/d/a)
    cols 5:12  7-tap conv weights (same center rule, dilation 3)
    col 12     ws[0,0]*0.5 (avg-branch weight, attn ch0; halved because the
               kernel feeds u1+u2 instead of (u1+u2)/2)
    col 13     ws[0,1] (max-branch weight, ch0)
    col 14     bs[0]
    col 15     ws[1,0]*0.5
    col 16     ws[1,1]
    col 17     bs[1]
    col 18     fusion_bias
    """
    gp = np.zeros((4, 32), np.float32)
    fb = float(np.asarray(inputs["fusion_bias"]).reshape(-1)[0])
    for g, n in enumerate(_GATE_ORDER):
        w0 = np.asarray(inputs[f"w{n}0"], np.float32)[0, 0]
        w1 = np.asarray(inputs[f"w{n}1"], np.float32)[0, 0]
        ws = np.asarray(inputs[f"w{n}s"], np.float32)[:, :, 0, 0]
        bs = np.asarray(inputs[f"b{n}s"], np.float32)
        along_h = n == "h"
        gp[g, 0:5] = w0[:, 2] if along_h else w0[2, :]
        gp[g, 5:12] = w1[:, 3] if along_h else w1[3, :]
        gp[g, 12] = ws[0, 0] * 0.5
        gp[g, 13] = ws[0, 1]
        gp[g, 14] = bs[0]
        gp[g, 15] = ws[1, 0] * 0.5
        gp[g, 16] = ws[1, 1]
        gp[g, 17] = bs[1]
        gp[g, 18] = fb
    return gp


def _emit(tc, outs, ins):
    import concourse.bass as bass
    import concourse.mybir as mybir

    F32 = mybir.dt.float32
    BF16 = mybir.dt.bfloat16
    AF = mybir.ActivationFunctionType
    OP = mybir.AluOpType

    nc = tc.nc
    x, gp = ins
    oh, ov = outs

    with ExitStack() as ctx:
        const = ctx.enter_context(tc.tile_pool(name="const", bufs=1))
        xpool = ctx.enter_context(tc.tile_pool(name="xp", bufs=1))
        small = ctx.enter_context(tc.tile_pool(name="small", bufs=1))
        gmaps = ctx.enter_context(tc.tile_pool(name="gmaps", bufs=1))
        res = ctx.enter_context(tc.tile_pool(name="res", bufs=4))
        g2p = ctx.enter_context(tc.tile_pool(name="g2p", bufs=2))
        g4p = ctx.enter_context(tc.tile_pool(name="g4p", bufs=2))
        f2p = ctx.enter_context(tc.tile_pool(name="f2p", bufs=2))
        stpool = ctx.enter_context(tc.tile_pool(name="stp", bufs=2))
        psum = ctx.enter_context(
            tc.tile_pool(name="ps", bufs=1, space=bass.MemorySpace.PSUM)
        )
        psumg = ctx.enter_context(
            tc.tile_pool(name="psg", bufs=2, space=bass.MemorySpace.PSUM)
        )

        # ---- params / constants (emitted first so they schedule early) ----
        gpt = const.tile([4, 32], F32)
        nc.sync.dma_start(gpt[:], gp[:])
        ones1b = const.tile([128, 1], BF16)
        nc.vector.memset(ones1b[:], 1.0 / 65536.0)
        # binary diagonal / anti-diagonal masks, built on idle GPSIMD time
        ones64 = const.tile([64, 64], F32)
        nc.vector.memset(ones64[:], 1.0)
        mskD = const.tile([64, 64], F32)
        mskA = const.tile([64, 64], F32)
        nc.gpsimd.affine_select(
            mskD[:], ones64[:], [[1, 64]], OP.is_equal, 0.0,
            base=0, channel_multiplier=-1,
        )
        nc.gpsimd.affine_select(
            mskA[:], ones64[:], [[1, 64]], OP.is_equal, 0.0,
            base=-63, channel_multiplier=1,
        )

        # PSUM accumulators, both on partition 0.
        psumS = psum.tile([1, 384], F32)  # [diag*64 | anti*64 | g16 (64x4)]
        psumV = psum.tile([1, 2048], F32)  # f2 channel sums (32 hg x 64 w)

        # force the Sigmoid ACT table to load during the idle in-phase
        # rather than on the gate critical path
        sigwarm = const.tile([1, 1], F32)
        nc.scalar.activation(sigwarm[:], gpt[0:1, 0:1], AF.Sigmoid)

        # ---- stream x in; per-tile work spread over DVE/ACT/GPS/PE ----
        xt = []
        for i in range(NT):
            t = xpool.tile([P, S], BF16, tag=f"x{i}", name=f"xt{i}")
            xt.append(t)
            eng = nc.sync if i % 2 == 0 else nc.scalar
            eng.dma_start(t[:], x[i * P : (i + 1) * P, :])
            x3 = t[:].rearrange("p (h w) -> p h w", h=HW)
            st = stpool.tile([P, 384], BF16, tag="st", name=f"st{i}")
            # diag / anti-diag gathers, pre-scaled by 64 (ACT; exact in bf16)
            nc.scalar.mul(st[:, 0:64], t[:, 0 : S : HW + 1], 64.0)
            nc.scalar.mul(st[:, 64:128], t[:, HW - 1 : S - HW + 1 : HW - 1], 64.0)
            # h path: fold w by 16 (pair-folds on DVE, then GPSIMD folds into
            # the stats tile). Sums only feed per-h means, so grouping within
            # each w run is free.
            g2 = g2p.tile([P, 2048], BF16, tag="g2", name=f"g2_{i}")
            g23 = g2[:].rearrange("p (h w) -> p h w", h=HW)
            nc.vector.tensor_tensor(
                g23, x3[:, :, 0:64:2], x3[:, :, 1:64:2], OP.add
            )
            g4 = g4p.tile([P, 1024], BF16, tag="g4", name=f"g4_{i}")
            g43 = g4[:].rearrange("p (h w) -> p h w", h=HW)
            nc.vector.tensor_tensor(
                g43, g23[:, :, 0:32:2], g23[:, :, 1:32:2], OP.add
            )
            g8 = g4p.tile([P, 512], BF16, tag="g8", name=f"g8_{i}")
            g83 = g8[:].rearrange("p (h w) -> p h w", h=HW)
            nc.gpsimd.tensor_tensor(
                g83, g43[:, :, 0:16:2], g43[:, :, 1:16:2], OP.add
            )
            st16 = st[:, 128:384].rearrange("p (h w) -> p h w", h=HW)
            nc.gpsimd.tensor_tensor(
                st16, g83[:, :, 0:8:2], g83[:, :, 1:8:2], OP.add
            )
            # v path: fold top/bottom h halves (contiguous reads on DVE; the
            # h grouping is irrelevant for the per-w mean), PE reduces over
            # channels
            f2 = f2p.tile([P, 2048], BF16, tag="f2", name=f"f2_{i}")
            nc.vector.tensor_tensor(f2[:], t[:, 0:2048], t[:, 2048:4096], OP.add)
            # channel reductions on PE (bf16 single pass, shared weights;
            # matmul PSUM output is capped at 512 f32 cols per instruction)
            nc.tensor.matmul(
                psumS[:], ones1b[:], st[:], start=(i == 0), stop=(i == NT - 1)
            )
            for j in range(4):
                sl = slice(j * 512, (j + 1) * 512)
                nc.tensor.matmul(
                    psumV[0:1, sl],
                    ones1b[:],
                    f2[:, sl],
                    start=(i == 0),
                    stop=(i == NT - 1),
                )

        # ---- tail: R = [m_v | m_h | m_d | m_a] on partition 0, one DMA
        # scatter into M4 [4, 64] (row g = gate g mean) ----
        M4 = small.tile([4, 64], F32)
        R = small.tile([1, 256], F32)
        nc.scalar.mul(R[0:1, 128:256], psumS[0:1, 0:128], 1.0)
        pv3 = psumV[0:1, :].rearrange("p (g w) -> p w g", g=32)
        nc.vector.reduce_sum(R[0:1, 0:64], pv3, axis=mybir.AxisListType.X)
        ph3 = psumS[0:1, 128:384].rearrange("p (h w) -> p h w", h=HW)
        nc.vector.reduce_sum(R[0:1, 64:128], ph3, axis=mybir.AxisListType.X)
        nc.sync.dma_start(M4[:], R[:])

        # ---- four gates on [4, 64]; row g = gate g ----
        def conv1d(dst, src, tap_base, ntaps, dil):
            c = ntaps // 2
            nc.vector.tensor_scalar(
                dst, src, gpt[:, tap_base + c : tap_base + c + 1], None, OP.mult
            )
            for k in range(ntaps):
                if k == c:
                    continue
                off = dil * (k - c)
                a0, b0 = max(0, -off), min(HW, HW - off)
                nc.vector.scalar_tensor_tensor(
                    dst[:, a0:b0],
                    src[:, a0 + off : b0 + off],
                    gpt[:, tap_base + k : tap_base + k + 1],
                    dst[:, a0:b0],
                    OP.mult,
                    OP.add,
                )

        u1 = small.tile([4, 64], F32)
        u2 = small.tile([4, 64], F32)
        conv1d(u1[:], M4[:], 0, 5, 1)
        conv1d(u2[:], u1[:], 5, 7, 3)

        sm = small.tile([4, 64], F32)  # u1+u2; the 0.5 lives in gp cols 12/15
        mx = small.tile([4, 64], F32)
        nc.vector.tensor_add(sm[:], u1[:], u2[:])
        nc.vector.tensor_tensor(mx[:], u1[:], u2[:], OP.max)
        z0 = small.tile([4, 64], F32)
        z1 = small.tile([4, 64], F32)
        nc.vector.tensor_scalar(z0[:], sm[:], gpt[:, 12:13], None, OP.mult)
        nc.vector.scalar_tensor_tensor(
            z0[:], mx[:], gpt[:, 13:14], z0[:], OP.mult, OP.add
        )
        nc.vector.tensor_scalar(z1[:], sm[:], gpt[:, 15:16], None, OP.mult)
        nc.vector.scalar_tensor_tensor(
            z1[:], mx[:], gpt[:, 16:17], z1[:], OP.mult, OP.add
        )
        at0 = small.tile([4, 64], F32)
        at1 = small.tile([4, 64], F32)
        nc.scalar.activation(at0[:], z0[:], AF.Sigmoid, bias=gpt[:, 14:15])
        nc.scalar.activation(at1[:], z1[:], AF.Sigmoid, bias=gpt[:, 17:18])
        nc.vector.tensor_mul(at0[:], u1[:], at0[:])
        nc.vector.tensor_mul(at1[:], u2[:], at1[:])
        nc.vector.tensor_add(at0[:], at0[:], at1[:])
        attn = small.tile([4, 64], F32)
        nc.scalar.activation(attn[:], at0[:], AF.Sigmoid)
        attnfb = small.tile([4, 64], F32)  # attn * fusion_bias (rows 2,3 used)
        nc.vector.tensor_scalar(attnfb[:], attn[:], gpt[:, 18:19], None, OP.mult)

        # ---- gain maps as [64, 64] bf16 partition-tiles (partition = h) ----
        ah_col = small.tile([64, 1], F32)
        fbd_col = small.tile([64, 1], F32)
        fba_col = small.tile([64, 1], F32)
        av = small.tile([1, 64], F32)
        avr = small.tile([64, 64], F32)
        nc.sync.dma_start(ah_col[:], attn[1:2, :])
        nc.sync.dma_start(fbd_col[:], attnfb[2:3, :])
        nc.scalar.dma_start(fba_col[:], attnfb[3:4, :])
        nc.scalar.dma_start(av[:], attn[0:1, :])
        nc.gpsimd.partition_broadcast(avr[:], av[:])

        # sum2d = fb*attn_d on diag + fb*attn_a on anti-diag (via 0/1 masks)
        sum2d = small.tile([64, 64], F32)
        nc.vector.tensor_scalar(sum2d[:], mskD[:], fbd_col[:], None, OP.mult)
        nc.vector.scalar_tensor_tensor(
            sum2d[:], mskA[:], fba_col[:], sum2d[:], OP.mult, OP.add
        )
        gh2d = small.tile([64, 64], BF16)
        gv2d = small.tile([64, 64], BF16)
        nc.vector.tensor_scalar(gh2d[:], sum2d[:], 1.0, ah_col[:], OP.add, OP.mult)
        nc.vector.scalar_tensor_tensor(
            gv2d[:], sum2d[:], 1.0, avr[:], OP.add, OP.mult
        )

        # flatten each map to a [1, 4096] row, then partition-broadcast:
        # G_h via PE outer-product into PSUM + ACT psum->sbuf bf16 copies,
        # G_v via GPSIMD partition_broadcast — disjoint engines, both idle
        # here, so the two maps materialize concurrently without stalling
        # the DVE multiplies that follow.
        ghrow = small.tile([1, S], BF16)
        gvrow = small.tile([1, S], BF16)
        nc.sync.dma_start(ghrow[:], gh2d[:])
        nc.scalar.dma_start(gvrow[:], gv2d[:])
        ones128 = const.tile([1, 128], BF16)
        nc.vector.memset(ones128[:], 1.0)
        G_h = gmaps.tile([P, S], BF16)
        G_v = gmaps.tile([P, S], BF16)
        for j in range(8):
            sl = slice(j * 512, (j + 1) * 512)
            pg = psumg.tile([P, 512], F32, tag="pg", name=f"pg{j}")
            nc.tensor.matmul(pg[:], ones128[:], ghrow[0:1, sl], start=True, stop=True)
            nc.scalar.mul(G_h[:, sl], pg[:], 1.0)
        for j in range(4):
            sl = slice(j * 1024, (j + 1) * 1024)
            nc.gpsimd.partition_broadcast(G_v[:, sl], gvrow[0:1, sl])

        # ---- out phase: out = x * G on DVE in [128, 2048] halves chasing
        # the arriving G chunks; each [128, 4096] bf16 tile goes out as one
        # 1 MiB DMA with 8 KiB lines ----
        for i in range(NT):
            osl = slice(i * P, (i + 1) * P)
            rh = res.tile([P, S], BF16, tag="res", name=f"rh{i}")
            for j in range(2):
                sl = slice(j * 2048, (j + 1) * 2048)
                nc.vector.tensor_mul(rh[:, sl], xt[i][:, sl], G_h[:, sl])
            nc.sync.dma_start(oh[osl, :], rh[:])
            rv = res.tile([P, S], BF16, tag="res", name=f"rv{i}")
            for j in range(2):
                sl = slice(j * 2048, (j + 1) * 2048)
                nc.vector.tensor_mul(rv[:, sl], xt[i][:, sl], G_v[:, sl])
            nc.scalar.dma_start(ov[osl, :], rv[:])


def _build_device_kernel():
    import concourse.bacc as bacc
    import concourse.mybir as mybir
    import concourse.tile as tile

    BF16 = mybir.dt.bfloat16
    F32 = mybir.dt.float32
    nc = bacc.Bacc("TRN2", target_bir_lowering=False, debug=False)
    x = nc.dram_tensor("x", [C, S], BF16, kind="ExternalInput").ap()
    gp = nc.dram_tensor("gp", [4, 32], F32, kind="ExternalInput").ap()
    oh = nc.dram_tensor("out_h", [C, S], BF16, kind="ExternalOutput").ap()
    ov = nc.dram_tensor("out_v", [C, S], BF16, kind="ExternalOutput").ap()

    with tile.TileContext(nc) as tc:
        _emit(tc, [oh, ov], [x, gp])

    nc.compile()
    return nc


def _get_nc():
    if "nc" not in _CACHE:
        _CACHE["nc"] = _build_device_kernel()
    return _CACHE["nc"]


def _run(inputs, **spmd_kwargs):
    """Shard, execute on 8 cores, gather. Returns (out_h, out_v, results)."""
    import ml_dtypes

    from concourse.bass_utils import run_bass_kernel_spmd

    nc = _get_nc()
    x = np.asarray(inputs["x"], dtype=np.float32)
    assert x.shape == (B, C, HW, HW), x.shape
    xb = np.ascontiguousarray(x.reshape(B, C, S).astype(ml_dtypes.bfloat16))
    gp = _pack_gate_params(inputs)
    in_maps = [{"x": xb[b], "gp": gp} for b in range(B)]
    r = run_bass_kernel_spmd(nc, in_maps, core_ids=list(range(B)), **spmd_kwargs)
    oh = np.stack([r.results[b]["out_h"] for b in range(B)])
    ov = np.stack([r.results[b]["out_v"] for b in range(B)])
    oh = oh.astype(np.float32).reshape(B, C, HW, HW)
    ov = ov.astype(np.float32).reshape(B, C, HW, HW)
    return oh, ov, r


def kernel(**inputs):
    oh, ov, _ = _run(inputs)
    return oh, ov


# revision 24
# speedup vs baseline: 1.6239x; 1.0536x over previous
"""Trainium2 Bass kernel for the DSAB block (nn_DSAB_block_61366492725647).

Contract: kernel(**inputs) takes the FULL unsharded inputs
(x: [8, 1024, 64, 64] f32 plus the 17 gate-weight tensors) and returns the
full output tuple (out_h, out_v), each [8, 1024, 64, 64] f32.

Strategy: data-parallel over batch B=8 across the 8 NeuronCores. Gate weights
are tiny and get host-packed into one [4, 32] tensor replicated to all cores.

This problem is memory-bound (per core: read x 16 MiB, write 32 MiB at f32).
The harness tolerance is 2e-2, so all bulk HBM traffic runs in bf16
(x in, out_h/out_v out: 25.2 MB/core total, ~0.6% worst-case rounding),
halving the DMA floor to ~70 us. Host converts f32<->bf16.

Per-core device kernel (x_b viewed [C=1024, S=4096] bf16, channels on
partitions). Measured engine rates that shaped the design: DVE tensor ops
~0.6 ns/elem in bf16 but tensor_reduce ~1.07 ns/elem regardless of dtype;
GPSIMD ~2 ns/elem; PE ~0.53 us per 512-col bf16 matmul; HWDGE rings move
~125/175/210 GB/s at 4/8/16 KiB lines. Hence: no big DVE reductions in the
streaming loop (PE reduces everything over channels), and all wide DMAs use
8 KiB lines.

  1. Stream x in as 8 tiles of [128, 4096] bf16 on both HWDGE rings
     (~2.9 us/tile cadence). Per tile:
       DVE:    g2 = w-pair fold (bf16), g8 = w-fold of g4 written straight
               into the stats tile, f2 = h-pair fold for the v path.
       GPSIMD: g4 = w-fold of g2.
       ACT:    diag / anti-diag gathers scaled by 64 (exact in bf16).
       PE:     one matmul reduces the stats tile [diag | anti | g8] over
               channels into psumS[1, 640]; four matmuls reduce f2 into
               psumV[1, 2048]. All bf16 single-pass with shared weights.
  2. Tail: R[1, 256] = [m_v | m_h | m_d | m_a] assembled on partition 0
     (DVE reduces psumV / psumS-g8 slices, ACT copies the gather sums),
     then one DMA scatters R to M4's four partitions (gate order v,h,d,a —
     matmul PSUM writes need base partition 0, so v's path stays there).
  3. Four LSK attention gates on [4, 64] tiles, conv taps as per-partition
     scalars (host-packed center row/col of the 5x5 / dilated 7x7).
  4. Gain maps G_h = attn_h * scale, G_v = attn_v * scale
     (scale = 1 + fusion_bias * diag projections) built as [64, 64] bf16
     partition-tiles from affine_select diagonal masks, flattened to a
     [1, 4096] row by DMA, then partition-broadcast by a single
     stride-0-source DMA per map ([128, 4096], one per ring) — the rings
     are idle at the phase boundary and this avoids GPSIMD broadcasts that
     stall concurrent DVE reads.
  5. out = x * G in [128, 4096] bf16 tiles on DVE (2.4 us each), written
     out as 1 MiB DMAs (8 KiB lines) alternating across both rings.
"""

from contextlib import ExitStack

import numpy as np

P = 128
C = 1024
HW = 64
S = HW * HW  # 4096
NT = C // P  # 8
B = 8

_CACHE = {}

# v first: the v-path PSUM accumulator must sit at base partition 0 (matmul
# PSUM writes only allow base 0/32/64), and with gate v on row 0 the gate
# means assemble contiguously on partition 0 as [m_v | m_h | m_d | m_a].
_GATE_ORDER = ("v", "h", "d", "a")


def _pack_gate_params(inputs):
    """Pack per-gate params into [4, 32] f32, one gate per row (v, h, d, a).

    cols 0:5   5-tap conv weights (center column of the 5x5 for the h gate,
               which convolves along H; center row for v/d/a)
    cols 5:12  7-tap conv weights (same center rule, dilation 3)
    col 12     ws[0,0]*0.5 (avg-branch weight, attn ch0; halved because the
               kernel feeds u1+u2 instead of (u1+u2)/2)
    col 13     ws[0,1] (max-branch weight, ch0)
    col 14     bs[0]
    col 15     ws[1,0]*0.5
    col 16     ws[1,1]
    col 17     bs[1]
    col 18     fusion_bias
    """
    gp = np.zeros((4, 32), np.float32)
    fb = float(np.asarray(inputs["fusion_bias"]).reshape(-1)[0])
    for g, n in enumerate(_GATE_ORDER):
        w0 = np.asarray(inputs[f"w{n}0"], np.float32)[0, 0]
        w1 = np.asarray(inputs[f"w{n}1"], np.float32)[0, 0]
        ws = np.asarray(inputs[f"w{n}s"], np.float32)[:, :, 0, 0]
        bs = np.asarray(inputs[f"b{n}s"], np.float32)
        along_h = n == "h"
        gp[g, 0:5] = w0[:, 2] if along_h else w0[2, :]
        gp[g, 5:12] = w1[:, 3] if along_h else w1[3, :]
        gp[g, 12] = ws[0, 0] * 0.5
        gp[g, 13] = ws[0, 1]
        gp[g, 14] = bs[0]
        gp[g, 15] = ws[1, 0] * 0.5
        gp[g, 16] = ws[1, 1]
        gp[g, 17] = bs[1]
        gp[g, 18] = fb
    return gp


def _emit(tc, outs, ins):
    import concourse.bass as bass
    import concourse.mybir as mybir

    F32 = mybir.dt.float32
    BF16 = mybir.dt.bfloat16
    AF = mybir.ActivationFunctionType
    OP = mybir.AluOpType

    nc = tc.nc
    x, gp = ins
    oh, ov = outs

    with ExitStack() as ctx:
        const = ctx.enter_context(tc.tile_pool(name="const", bufs=1))
        xpool = ctx.enter_context(tc.tile_pool(name="xp", bufs=1))
        small = ctx.enter_context(tc.tile_pool(name="small", bufs=1))
        gmaps = ctx.enter_context(tc.tile_pool(name="gmaps", bufs=1))
        res = ctx.enter_context(tc.tile_pool(name="res", bufs=4))
        g2p = ctx.enter_context(tc.tile_pool(name="g2p", bufs=2))
        g4p = ctx.enter_context(tc.tile_pool(name="g4p", bufs=2))
        f2p = ctx.enter_context(tc.tile_pool(name="f2p", bufs=2))
        stpool = ctx.enter_context(tc.tile_pool(name="stp", bufs=2))
        psum = ctx.enter_context(
            tc.tile_pool(name="ps", bufs=1, space=bass.MemorySpace.PSUM)
        )
        psumg = ctx.enter_context(
            tc.tile_pool(name="psg", bufs=2, space=bass.MemorySpace.PSUM)
        )
        psumgv = ctx.enter_context(
            tc.tile_pool(name="psgv", bufs=1, space=bass.MemorySpace.PSUM)
        )

        # ---- params / constants (emitted first so they schedule early) ----
        gpt = const.tile([4, 32], F32)
        nc.sync.dma_start(gpt[:], gp[:])
        ones1b = const.tile([128, 1], BF16)
        nc.vector.memset(ones1b[:], 1.0 / 65536.0)
        # binary diagonal / anti-diagonal masks, built on idle GPSIMD time
        ones64 = const.tile([64, 64], F32)
        nc.vector.memset(ones64[:], 1.0)
        mskD = const.tile([64, 64], F32)
        mskA = const.tile([64, 64], F32)
        nc.gpsimd.affine_select(
            mskD[:], ones64[:], [[1, 64]], OP.is_equal, 0.0,
            base=0, channel_multiplier=-1,
        )
        nc.gpsimd.affine_select(
            mskA[:], ones64[:], [[1, 64]], OP.is_equal, 0.0,
            base=-63, channel_multiplier=1,
        )

        # PSUM accumulators, both on partition 0.
        psumS = psum.tile([1, 384], F32)  # [diag*64 | anti*64 | g16 (64x4)]
        psumV = psum.tile([1, 2048], F32)  # f2 channel sums (32 hg x 64 w)

        # force the Sigmoid ACT table to load during the idle in-phase
        # rather than on the gate critical path
        sigwarm = const.tile([1, 1], F32)
        nc.scalar.activation(sigwarm[:], gpt[0:1, 0:1], AF.Sigmoid)

        # ---- stream x in; per-tile work spread over DVE/ACT/GPS/PE ----
        xt = []
        for i in range(NT):
            t = xpool.tile([P, S], BF16, tag=f"x{i}", name=f"xt{i}")
            xt.append(t)
            eng = nc.sync if i % 2 == 0 else nc.scalar
            eng.dma_start(t[:], x[i * P : (i + 1) * P, :])
            x3 = t[:].rearrange("p (h w) -> p h w", h=HW)
            st = stpool.tile([P, 384], BF16, tag="st", name=f"st{i}")
            # diag / anti-diag gathers, pre-scaled by 64 (ACT; exact in bf16)
            nc.scalar.mul(st[:, 0:64], t[:, 0 : S : HW + 1], 64.0)
            nc.scalar.mul(st[:, 64:128], t[:, HW - 1 : S - HW + 1 : HW - 1], 64.0)
            # h path: fold w by 16 via half-folds — contiguous inner runs
            # keep DVE in its 2x bf16 mode (stride-2 patterns halve it), and
            # the sums only feed per-h means so w grouping is free.
            g2 = g2p.tile([P, 2048], BF16, tag="g2", name=f"g2_{i}")
            g23 = g2[:].rearrange("p (h w) -> p h w", h=HW)
            nc.vector.tensor_tensor(
                g23, x3[:, :, 0:32], x3[:, :, 32:64], OP.add
            )
            g4 = g4p.tile([P, 1024], BF16, tag="g4", name=f"g4_{i}")
            g43 = g4[:].rearrange("p (h w) -> p h w", h=HW)
            nc.gpsimd.tensor_tensor(
                g43, g23[:, :, 0:16], g23[:, :, 16:32], OP.add
            )
            g8 = g4p.tile([P, 512], BF16, tag="g8", name=f"g8_{i}")
            g83 = g8[:].rearrange("p (h w) -> p h w", h=HW)
            nc.vector.tensor_tensor(
                g83, g43[:, :, 0:8], g43[:, :, 8:16], OP.add
            )
            st16 = st[:, 128:384].rearrange("p (h w) -> p h w", h=HW)
            nc.vector.tensor_tensor(
                st16, g83[:, :, 0:4], g83[:, :, 4:8], OP.add
            )
            # v path: fold top/bottom h halves (contiguous reads on DVE; the
            # h grouping is irrelevant for the per-w mean), PE reduces over
            # channels
            f2 = f2p.tile([P, 2048], BF16, tag="f2", name=f"f2_{i}")
            nc.vector.tensor_tensor(f2[:], t[:, 0:2048], t[:, 2048:4096], OP.add)
            # channel reductions on PE (bf16 single pass, shared weights;
            # matmul PSUM output is capped at 512 f32 cols per instruction)
            nc.tensor.matmul(
                psumS[:], ones1b[:], st[:], start=(i == 0), stop=(i == NT - 1)
            )
            for j in range(4):
                sl = slice(j * 512, (j + 1) * 512)
                nc.tensor.matmul(
                    psumV[0:1, sl],
                    ones1b[:],
                    f2[:, sl],
                    start=(i == 0),
                    stop=(i == NT - 1),
                )

        # ---- tail: R = [m_v | m_h | m_d | m_a] on partition 0, one DMA
        # scatter into M4 [4, 64] (row g = gate g mean) ----
        M4 = small.tile([4, 64], F32)
        R = small.tile([1, 256], F32)
        nc.scalar.mul(R[0:1, 128:256], psumS[0:1, 0:128], 1.0)
        pv3 = psumV[0:1, :].rearrange("p (g w) -> p w g", g=32)
        nc.vector.reduce_sum(R[0:1, 0:64], pv3, axis=mybir.AxisListType.X)
        ph3 = psumS[0:1, 128:384].rearrange("p (h w) -> p h w", h=HW)
        nc.vector.reduce_sum(R[0:1, 64:128], ph3, axis=mybir.AxisListType.X)
        nc.sync.dma_start(M4[:], R[:])

        # ---- four gates on [4, 64]; row g = gate g ----
        def conv1d(dst, src, tap_base, ntaps, dil):
            c = ntaps // 2
            nc.vector.tensor_scalar(
                dst, src, gpt[:, tap_base + c : tap_base + c + 1], None, OP.mult
            )
            for k in range(ntaps):
                if k == c:
                    continue
                off = dil * (k - c)
                a0, b0 = max(0, -off), min(HW, HW - off)
                nc.vector.scalar_tensor_tensor(
                    dst[:, a0:b0],
                    src[:, a0 + off : b0 + off],
                    gpt[:, tap_base + k : tap_base + k + 1],
                    dst[:, a0:b0],
                    OP.mult,
                    OP.add,
                )

        u1 = small.tile([4, 64], F32)
        u2 = small.tile([4, 64], F32)
        conv1d(u1[:], M4[:], 0, 5, 1)
        conv1d(u2[:], u1[:], 5, 7, 3)

        sm = small.tile([4, 64], F32)  # u1+u2; the 0.5 lives in gp cols 12/15
        mx = small.tile([4, 64], F32)
        nc.vector.tensor_add(sm[:], u1[:], u2[:])
        nc.vector.tensor_tensor(mx[:], u1[:], u2[:], OP.max)
        z0 = small.tile([4, 64], F32)
        z1 = small.tile([4, 64], F32)
        nc.vector.tensor_scalar(z0[:], sm[:], gpt[:, 12:13], None, OP.mult)
        nc.vector.scalar_tensor_tensor(
            z0[:], mx[:], gpt[:, 13:14], z0[:], OP.mult, OP.add
        )
        nc.vector.tensor_scalar(z1[:], sm[:], gpt[:, 15:16], None, OP.mult)
        nc.vector.scalar_tensor_tensor(
            z1[:], mx[:], gpt[:, 16:17], z1[:], OP.mult, OP.add
        )
        at0 = small.tile([4, 64], F32)
        at1 = small.tile([4, 64], F32)
        nc.scalar.activation(at0[:], z0[:], AF.Sigmoid, bias=gpt[:, 14:15])
        nc.scalar.activation(at1[:], z1[:], AF.Sigmoid, bias=gpt[:, 17:18])
        nc.vector.tensor_mul(at0[:], u1[:], at0[:])
        nc.vector.tensor_mul(at1[:], u2[:], at1[:])
        nc.vector.tensor_add(at0[:], at0[:], at1[:])
        attn = small.tile([4, 64], F32)
        nc.scalar.activation(attn[:], at0[:], AF.Sigmoid)
        attnfb = small.tile([4, 64], F32)  # attn * fusion_bias (rows 2,3 used)
        nc.vector.tensor_scalar(attnfb[:], attn[:], gpt[:, 18:19], None, OP.mult)

        # ---- gain maps as [64, 64] bf16 partition-tiles (partition = h) ----
        ah_col = small.tile([64, 1], F32)
        fbd_col = small.tile([64, 1], F32)
        fba_col = small.tile([64, 1], F32)
        av = small.tile([1, 64], F32)
        avr = small.tile([64, 64], F32)
        nc.sync.dma_start(ah_col[:], attn[1:2, :])
        nc.sync.dma_start(fbd_col[:], attnfb[2:3, :])
        nc.scalar.dma_start(fba_col[:], attnfb[3:4, :])
        nc.scalar.dma_start(av[:], attn[0:1, :])
        nc.gpsimd.partition_broadcast(avr[:], av[:])

        # sum2d = fb*attn_d on diag + fb*attn_a on anti-diag (via 0/1 masks)
        sum2d = small.tile([64, 64], F32)
        nc.vector.tensor_scalar(sum2d[:], mskD[:], fbd_col[:], None, OP.mult)
        nc.vector.scalar_tensor_tensor(
            sum2d[:], mskA[:], fba_col[:], sum2d[:], OP.mult, OP.add
        )
        gh2d = small.tile([64, 64], BF16)
        gv2d = small.tile([64, 64], BF16)
        nc.vector.tensor_scalar(gh2d[:], sum2d[:], 1.0, ah_col[:], OP.add, OP.mult)
        nc.vector.scalar_tensor_tensor(
            gv2d[:], sum2d[:], 1.0, avr[:], OP.add, OP.mult
        )

        # flatten each map to a [1, 4096] row, then partition-broadcast:
        # G_h via PE outer-product into PSUM + ACT psum->sbuf bf16 copies,
        # G_v via GPSIMD partition_broadcast — disjoint engines, both idle
        # here, so the two maps materialize concurrently without stalling
        # the DVE multiplies that follow.
        ghrow = small.tile([1, S], BF16)
        gvrow = small.tile([1, S], BF16)
        nc.sync.dma_start(ghrow[:], gh2d[:])
        nc.scalar.dma_start(gvrow[:], gv2d[:])
        ones128 = const.tile([1, 128], BF16)
        nc.vector.memset(ones128[:], 1.0)
        G_h = gmaps.tile([P, S], BF16)
        G_v = gmaps.tile([P, S], BF16)
        for j in range(8):
            sl = slice(j * 512, (j + 1) * 512)
            pg = psumg.tile([P, 512], F32, tag="pg", name=f"pg{j}")
            nc.tensor.matmul(pg[:], ones128[:], ghrow[0:1, sl], start=True, stop=True)
            nc.scalar.mul(G_h[:, sl], pg[:], 1.0)
        pgv = []
        for j in range(8):
            sl = slice(j * 512, (j + 1) * 512)
            pg = psumgv.tile([P, 512], F32, tag="pgv", name=f"pgv{j}")
            pgv.append(pg)
            nc.tensor.matmul(pg[:], ones128[:], gvrow[0:1, sl], start=True, stop=True)

        # ---- out phase: all out_h tiles first (G_h is ready ~5 us before
        # G_v), DMAs alternating rings; G_v's psum->sbuf copies hide between
        # the h multiplies on DVE; then the out_v tiles ----
        for i in range(NT):
            osl = slice(i * P, (i + 1) * P)
            rh = res.tile([P, S], BF16, tag="res", name=f"rh{i}")
            for j in range(2):
                sl = slice(j * 2048, (j + 1) * 2048)
                nc.vector.tensor_mul(rh[:, sl], xt[i][:, sl], G_h[:, sl])
            if i < NT - 1:
                sl = slice(i * 512, (i + 1) * 512)
                nc.vector.tensor_copy(G_v[:, sl], pgv[i][:])
            eng = nc.sync if i % 2 == 0 else nc.scalar
            eng.dma_start(oh[osl, :], rh[:])
        sl = slice(7 * 512, 8 * 512)
        nc.vector.tensor_copy(G_v[:, sl], pgv[7][:])
        for i in range(NT):
            osl = slice(i * P, (i + 1) * P)
            rv = res.tile([P, S], BF16, tag="res", name=f"rv{i}")
            for j in range(2):
                sl = slice(j * 2048, (j + 1) * 2048)
                nc.vector.tensor_mul(rv[:, sl], xt[i][:, sl], G_v[:, sl])
            eng = nc.sync if i % 2 == 0 else nc.scalar
            eng.dma_start(ov[osl, :], rv[:])


def _build_device_kernel():
    import concourse.bacc as bacc
    import concourse.mybir as mybir
    import concourse.tile as tile

    BF16 = mybir.dt.bfloat16
    F32 = mybir.dt.float32
    nc = bacc.Bacc("TRN2", target_bir_lowering=False, debug=False)
    x = nc.dram_tensor("x", [C, S], BF16, kind="ExternalInput").ap()
    gp = nc.dram_tensor("gp", [4, 32], F32, kind="ExternalInput").ap()
    oh = nc.dram_tensor("out_h", [C, S], BF16, kind="ExternalOutput").ap()
    ov = nc.dram_tensor("out_v", [C, S], BF16, kind="ExternalOutput").ap()

    with tile.TileContext(nc) as tc:
        _emit(tc, [oh, ov], [x, gp])

    nc.compile()
    return nc


def _get_nc():
    if "nc" not in _CACHE:
        _CACHE["nc"] = _build_device_kernel()
    return _CACHE["nc"]


def _run(inputs, **spmd_kwargs):
    """Shard, execute on 8 cores, gather. Returns (out_h, out_v, results)."""
    import ml_dtypes

    from concourse.bass_utils import run_bass_kernel_spmd

    nc = _get_nc()
    x = np.asarray(inputs["x"], dtype=np.float32)
    assert x.shape == (B, C, HW, HW), x.shape
    xb = np.ascontiguousarray(x.reshape(B, C, S).astype(ml_dtypes.bfloat16))
    gp = _pack_gate_params(inputs)
    in_maps = [{"x": xb[b], "gp": gp} for b in range(B)]
    r = run_bass_kernel_spmd(nc, in_maps, core_ids=list(range(B)), **spmd_kwargs)
    oh = np.stack([r.results[b]["out_h"] for b in range(B)])
    ov = np.stack([r.results[b]["out_v"] for b in range(B)])
    oh = oh.astype(np.float32).reshape(B, C, HW, HW)
    ov = ov.astype(np.float32).reshape(B, C, HW, HW)
    return oh, ov, r


def kernel(**inputs):
    oh, ov, _ = _run(inputs)
    return oh, ov


# revision 25
# speedup vs baseline: 1.6695x; 1.0281x over previous
"""Trainium2 Bass kernel for the DSAB block (nn_DSAB_block_61366492725647).

Contract: kernel(**inputs) takes the FULL unsharded inputs
(x: [8, 1024, 64, 64] f32 plus the 17 gate-weight tensors) and returns the
full output tuple (out_h, out_v), each [8, 1024, 64, 64] f32.

Strategy: data-parallel over batch B=8 across the 8 NeuronCores. Gate weights
are tiny and get host-packed into one [4, 32] tensor replicated to all cores.

This problem is memory-bound (per core: read x 16 MiB, write 32 MiB at f32).
The harness tolerance is 2e-2, so all bulk HBM traffic runs in bf16
(x in, out_h/out_v out: 25.2 MB/core total, ~0.6% worst-case rounding),
halving the DMA floor to ~70 us. Host converts f32<->bf16.

Per-core device kernel (x_b viewed [C=1024, S=4096] bf16, channels on
partitions). Measured engine rates that shaped the design: DVE tensor ops
~0.6 ns/elem in bf16 but tensor_reduce ~1.07 ns/elem regardless of dtype;
GPSIMD ~2 ns/elem; PE ~0.53 us per 512-col bf16 matmul; HWDGE rings move
~125/175/210 GB/s at 4/8/16 KiB lines. Hence: no big DVE reductions in the
streaming loop (PE reduces everything over channels), and all wide DMAs use
8 KiB lines.

  1. Stream x in as 8 tiles of [128, 4096] bf16 on both HWDGE rings
     (~2.9 us/tile cadence). Per tile:
       DVE:    g2 = w-pair fold (bf16), g8 = w-fold of g4 written straight
               into the stats tile, f2 = h-pair fold for the v path.
       GPSIMD: g4 = w-fold of g2.
       ACT:    diag / anti-diag gathers scaled by 64 (exact in bf16).
       PE:     one matmul reduces the stats tile [diag | anti | g8] over
               channels into psumS[1, 640]; four matmuls reduce f2 into
               psumV[1, 2048]. All bf16 single-pass with shared weights.
  2. Tail: R[1, 256] = [m_v | m_h | m_d | m_a] assembled on partition 0
     (DVE reduces psumV / psumS-g8 slices, ACT copies the gather sums),
     then one DMA scatters R to M4's four partitions (gate order v,h,d,a —
     matmul PSUM writes need base partition 0, so v's path stays there).
  3. Four LSK attention gates on [4, 64] tiles, conv taps as per-partition
     scalars (host-packed center row/col of the 5x5 / dilated 7x7).
  4. Gain maps G_h = attn_h * scale, G_v = attn_v * scale
     (scale = 1 + fusion_bias * diag projections) built as [64, 64] bf16
     partition-tiles from affine_select diagonal masks, flattened to a
     [1, 4096] row by DMA, then partition-broadcast by a single
     stride-0-source DMA per map ([128, 4096], one per ring) — the rings
     are idle at the phase boundary and this avoids GPSIMD broadcasts that
     stall concurrent DVE reads.
  5. out = x * G in [128, 4096] bf16 tiles on DVE (2.4 us each), written
     out as 1 MiB DMAs (8 KiB lines) alternating across both rings.
"""

from contextlib import ExitStack

import numpy as np

P = 128
C = 1024
HW = 64
S = HW * HW  # 4096
NT = C // P  # 8
B = 8

_CACHE = {}

# v first: the v-path PSUM accumulator must sit at base partition 0 (matmul
# PSUM writes only allow base 0/32/64), and with gate v on row 0 the gate
# means assemble contiguously on partition 0 as [m_v | m_h | m_d | m_a].
_GATE_ORDER = ("v", "h", "d", "a")


def _pack_gate_params(inputs):
    """Pack per-gate params into [4, 32] f32, one gate per row (v, h, d, a).

    cols 0:5   5-tap conv weights (center column of the 5x5 for the h gate,
               which convolves along H; center row for v/d/a)
    cols 5:12  7-tap conv weights (same center rule, dilation 3)
    col 12     ws[0,0]*0.5 (avg-branch weight, attn ch0; halved because the
               kernel feeds u1+u2 instead of (u1+u2)/2)
    col 13     ws[0,1] (max-branch weight, ch0)
    col 14     bs[0]
    col 15     ws[1,0]*0.5
    col 16     ws[1,1]
    col 17     bs[1]
    col 18     fusion_bias
    """
    gp = np.zeros((4, 32), np.float32)
    fb = float(np.asarray(inputs["fusion_bias"]).reshape(-1)[0])
    for g, n in enumerate(_GATE_ORDER):
        w0 = np.asarray(inputs[f"w{n}0"], np.float32)[0, 0]
        w1 = np.asarray(inputs[f"w{n}1"], np.float32)[0, 0]
        ws = np.asarray(inputs[f"w{n}s"], np.float32)[:, :, 0, 0]
        bs = np.asarray(inputs[f"b{n}s"], np.float32)
        along_h = n == "h"
        gp[g, 0:5] = w0[:, 2] if along_h else w0[2, :]
        gp[g, 5:12] = w1[:, 3] if along_h else w1[3, :]
        gp[g, 12] = ws[0, 0] * 0.5
        gp[g, 13] = ws[0, 1]
        gp[g, 14] = bs[0]
        gp[g, 15] = ws[1, 0] * 0.5
        gp[g, 16] = ws[1, 1]
        gp[g, 17] = bs[1]
        gp[g, 18] = fb
    return gp


def _emit(tc, outs, ins):
    import concourse.bass as bass
    import concourse.mybir as mybir

    F32 = mybir.dt.float32
    BF16 = mybir.dt.bfloat16
    AF = mybir.ActivationFunctionType
    OP = mybir.AluOpType

    nc = tc.nc
    x, gp = ins
    oh, ov = outs

    with ExitStack() as ctx:
        const = ctx.enter_context(tc.tile_pool(name="const", bufs=1))
        xpool = ctx.enter_context(tc.tile_pool(name="xp", bufs=1))
        small = ctx.enter_context(tc.tile_pool(name="small", bufs=1))
        gmaps = ctx.enter_context(tc.tile_pool(name="gmaps", bufs=1))
        res = ctx.enter_context(tc.tile_pool(name="res", bufs=4))
        g2p = ctx.enter_context(tc.tile_pool(name="g2p", bufs=2))
        g4p = ctx.enter_context(tc.tile_pool(name="g4p", bufs=2))
        f2p = ctx.enter_context(tc.tile_pool(name="f2p", bufs=2))
        stpool = ctx.enter_context(tc.tile_pool(name="stp", bufs=2))
        psum = ctx.enter_context(
            tc.tile_pool(name="ps", bufs=1, space=bass.MemorySpace.PSUM)
        )
        psumg = ctx.enter_context(
            tc.tile_pool(name="psg", bufs=2, space=bass.MemorySpace.PSUM)
        )
        psumgv = ctx.enter_context(
            tc.tile_pool(name="psgv", bufs=1, space=bass.MemorySpace.PSUM)
        )

        # ---- params / constants (emitted first so they schedule early) ----
        gpt = const.tile([4, 32], F32)
        nc.sync.dma_start(gpt[:], gp[:])
        ones1b = const.tile([128, 1], BF16)
        nc.vector.memset(ones1b[:], 1.0 / 65536.0)
        # binary diagonal / anti-diagonal masks, built on idle GPSIMD time
        ones64 = const.tile([64, 64], F32)
        nc.vector.memset(ones64[:], 1.0)
        mskD = const.tile([64, 64], F32)
        mskA = const.tile([64, 64], F32)
        nc.gpsimd.affine_select(
            mskD[:], ones64[:], [[1, 64]], OP.is_equal, 0.0,
            base=0, channel_multiplier=-1,
        )
        nc.gpsimd.affine_select(
            mskA[:], ones64[:], [[1, 64]], OP.is_equal, 0.0,
            base=-63, channel_multiplier=1,
        )

        # PSUM accumulators, both on partition 0.
        psumS = psum.tile([1, 384], F32)  # [diag*64 | anti*64 | g16 (64x4)]
        psumV = psum.tile([1, 2048], F32)  # f2 channel sums (32 hg x 64 w)

        # force the Sigmoid ACT table to load during the idle in-phase
        # rather than on the gate critical path
        sigwarm = const.tile([1, 1], F32)
        nc.scalar.activation(sigwarm[:], gpt[0:1, 0:1], AF.Sigmoid)

        # ---- stream x in; per-tile work spread over DVE/ACT/GPS/PE.
        # The h path is software-pipelined: tile i's g8/st16 folds (which
        # depend on GPSIMD's g4) and its stats matmul are emitted during
        # iteration i+1, so the in-order DVE queue never stalls on GPSIMD.
        xt = []

        def finish_htail(i, g4, st):
            g43 = g4[:].rearrange("p (h w) -> p h w", h=HW)
            g8 = g4p.tile([P, 512], BF16, tag="g8", name=f"g8_{i}")
            g83 = g8[:].rearrange("p (h w) -> p h w", h=HW)
            nc.vector.tensor_tensor(g83, g43[:, :, 0:8], g43[:, :, 8:16], OP.add)
            st16 = st[:, 128:384].rearrange("p (h w) -> p h w", h=HW)
            nc.vector.tensor_tensor(st16, g83[:, :, 0:4], g83[:, :, 4:8], OP.add)
            nc.tensor.matmul(
                psumS[:], ones1b[:], st[:], start=(i == 0), stop=(i == NT - 1)
            )

        pend = None
        for i in range(NT):
            t = xpool.tile([P, S], BF16, tag=f"x{i}", name=f"xt{i}")
            xt.append(t)
            eng = nc.sync if i % 2 == 0 else nc.scalar
            eng.dma_start(t[:], x[i * P : (i + 1) * P, :])
            x3 = t[:].rearrange("p (h w) -> p h w", h=HW)
            st = stpool.tile([P, 384], BF16, tag="st", name=f"st{i}")
            # diag / anti-diag gathers, pre-scaled by 64 (ACT; exact in bf16)
            nc.scalar.mul(st[:, 0:64], t[:, 0 : S : HW + 1], 64.0)
            nc.scalar.mul(st[:, 64:128], t[:, HW - 1 : S - HW + 1 : HW - 1], 64.0)
            # h path: fold w by 16 via half-folds — contiguous inner runs
            # keep DVE in its 2x bf16 mode (stride-2 patterns halve it), and
            # the sums only feed per-h means so w grouping is free.
            g2 = g2p.tile([P, 2048], BF16, tag="g2", name=f"g2_{i}")
            g23 = g2[:].rearrange("p (h w) -> p h w", h=HW)
            nc.vector.tensor_tensor(
                g23, x3[:, :, 0:32], x3[:, :, 32:64], OP.add
            )
            # v path: fold top/bottom h halves (contiguous; the h grouping is
            # irrelevant for the per-w mean), PE reduces over channels
            f2 = f2p.tile([P, 2048], BF16, tag="f2", name=f"f2_{i}")
            nc.vector.tensor_tensor(f2[:], t[:, 0:2048], t[:, 2048:4096], OP.add)
            g4 = g4p.tile([P, 1024], BF16, tag="g4", name=f"g4_{i}")
            g43 = g4[:].rearrange("p (h w) -> p h w", h=HW)
            nc.gpsimd.tensor_tensor(
                g43, g23[:, :, 0:16], g23[:, :, 16:32], OP.add
            )
            for j in range(4):
                sl = slice(j * 512, (j + 1) * 512)
                nc.tensor.matmul(
                    psumV[0:1, sl],
                    ones1b[:],
                    f2[:, sl],
                    start=(i == 0),
                    stop=(i == NT - 1),
                )
            if pend is not None:
                finish_htail(*pend)
            pend = (i, g4, st)
        finish_htail(*pend)

        # ---- tail: R = [m_v | m_h | m_d | m_a] on partition 0, one DMA
        # scatter into M4 [4, 64] (row g = gate g mean) ----
        M4 = small.tile([4, 64], F32)
        R = small.tile([1, 256], F32)
        nc.scalar.mul(R[0:1, 128:256], psumS[0:1, 0:128], 1.0)
        pv3 = psumV[0:1, :].rearrange("p (g w) -> p w g", g=32)
        nc.vector.reduce_sum(R[0:1, 0:64], pv3, axis=mybir.AxisListType.X)
        ph3 = psumS[0:1, 128:384].rearrange("p (h w) -> p h w", h=HW)
        nc.vector.reduce_sum(R[0:1, 64:128], ph3, axis=mybir.AxisListType.X)
        nc.sync.dma_start(M4[:], R[:])

        # ---- four gates on [4, 64]; row g = gate g ----
        def conv1d(dst, src, tap_base, ntaps, dil):
            c = ntaps // 2
            nc.vector.tensor_scalar(
                dst, src, gpt[:, tap_base + c : tap_base + c + 1], None, OP.mult
            )
            for k in range(ntaps):
                if k == c:
                    continue
                off = dil * (k - c)
                a0, b0 = max(0, -off), min(HW, HW - off)
                nc.vector.scalar_tensor_tensor(
                    dst[:, a0:b0],
                    src[:, a0 + off : b0 + off],
                    gpt[:, tap_base + k : tap_base + k + 1],
                    dst[:, a0:b0],
                    OP.mult,
                    OP.add,
                )

        u1 = small.tile([4, 64], F32)
        u2 = small.tile([4, 64], F32)
        conv1d(u1[:], M4[:], 0, 5, 1)
        conv1d(u2[:], u1[:], 5, 7, 3)

        sm = small.tile([4, 64], F32)  # u1+u2; the 0.5 lives in gp cols 12/15
        mx = small.tile([4, 64], F32)
        nc.vector.tensor_add(sm[:], u1[:], u2[:])
        nc.vector.tensor_tensor(mx[:], u1[:], u2[:], OP.max)
        z0 = small.tile([4, 64], F32)
        z1 = small.tile([4, 64], F32)
        nc.vector.tensor_scalar(z0[:], sm[:], gpt[:, 12:13], None, OP.mult)
        nc.vector.scalar_tensor_tensor(
            z0[:], mx[:], gpt[:, 13:14], z0[:], OP.mult, OP.add
        )
        nc.vector.tensor_scalar(z1[:], sm[:], gpt[:, 15:16], None, OP.mult)
        nc.vector.scalar_tensor_tensor(
            z1[:], mx[:], gpt[:, 16:17], z1[:], OP.mult, OP.add
        )
        at0 = small.tile([4, 64], F32)
        at1 = small.tile([4, 64], F32)
        nc.scalar.activation(at0[:], z0[:], AF.Sigmoid, bias=gpt[:, 14:15])
        nc.scalar.activation(at1[:], z1[:], AF.Sigmoid, bias=gpt[:, 17:18])
        nc.vector.tensor_mul(at0[:], u1[:], at0[:])
        nc.vector.tensor_mul(at1[:], u2[:], at1[:])
        nc.vector.tensor_add(at0[:], at0[:], at1[:])
        attn = small.tile([4, 64], F32)
        nc.scalar.activation(attn[:], at0[:], AF.Sigmoid)
        attnfb = small.tile([4, 64], F32)  # attn * fusion_bias (rows 2,3 used)
        nc.vector.tensor_scalar(attnfb[:], attn[:], gpt[:, 18:19], None, OP.mult)

        # ---- gain maps as [64, 64] bf16 partition-tiles (partition = h) ----
        ah_col = small.tile([64, 1], F32)
        fbd_col = small.tile([64, 1], F32)
        fba_col = small.tile([64, 1], F32)
        av = small.tile([1, 64], F32)
        avr = small.tile([64, 64], F32)
        nc.sync.dma_start(ah_col[:], attn[1:2, :])
        nc.sync.dma_start(fbd_col[:], attnfb[2:3, :])
        nc.scalar.dma_start(fba_col[:], attnfb[3:4, :])
        nc.scalar.dma_start(av[:], attn[0:1, :])
        nc.gpsimd.partition_broadcast(avr[:], av[:])

        # sum2d = fb*attn_d on diag + fb*attn_a on anti-diag (via 0/1 masks)
        sum2d = small.tile([64, 64], F32)
        nc.vector.tensor_scalar(sum2d[:], mskD[:], fbd_col[:], None, OP.mult)
        nc.vector.scalar_tensor_tensor(
            sum2d[:], mskA[:], fba_col[:], sum2d[:], OP.mult, OP.add
        )
        gh2d = small.tile([64, 64], BF16)
        gv2d = small.tile([64, 64], BF16)
        nc.vector.tensor_scalar(gh2d[:], sum2d[:], 1.0, ah_col[:], OP.add, OP.mult)
        nc.vector.scalar_tensor_tensor(
            gv2d[:], sum2d[:], 1.0, avr[:], OP.add, OP.mult
        )

        # flatten each map to a [1, 4096] row, then partition-broadcast:
        # G_h via PE outer-product into PSUM + ACT psum->sbuf bf16 copies,
        # G_v via GPSIMD partition_broadcast — disjoint engines, both idle
        # here, so the two maps materialize concurrently without stalling
        # the DVE multiplies that follow.
        ghrow = small.tile([1, S], BF16)
        gvrow = small.tile([1, S], BF16)
        nc.sync.dma_start(ghrow[:], gh2d[:])
        nc.scalar.dma_start(gvrow[:], gv2d[:])
        ones128 = const.tile([1, 128], BF16)
        nc.vector.memset(ones128[:], 1.0)
        G_h = gmaps.tile([P, S], BF16)
        G_v = gmaps.tile([P, S], BF16)
        for j in range(8):
            sl = slice(j * 512, (j + 1) * 512)
            pg = psumg.tile([P, 512], F32, tag="pg", name=f"pg{j}")
            nc.tensor.matmul(pg[:], ones128[:], ghrow[0:1, sl], start=True, stop=True)
            nc.scalar.mul(G_h[:, sl], pg[:], 1.0)
        pgv = []
        for j in range(8):
            sl = slice(j * 512, (j + 1) * 512)
            pg = psumgv.tile([P, 512], F32, tag="pgv", name=f"pgv{j}")
            pgv.append(pg)
            nc.tensor.matmul(pg[:], ones128[:], gvrow[0:1, sl], start=True, stop=True)

        # ---- out phase: all out_h tiles first (G_h is ready ~5 us before
        # G_v), DMAs alternating rings; G_v's psum->sbuf copies hide between
        # the h multiplies on DVE; then the out_v tiles ----
        for i in range(NT):
            osl = slice(i * P, (i + 1) * P)
            rh = res.tile([P, S], BF16, tag="res", name=f"rh{i}")
            for j in range(2):
                sl = slice(j * 2048, (j + 1) * 2048)
                nc.vector.tensor_mul(rh[:, sl], xt[i][:, sl], G_h[:, sl])
            if i < NT - 1:
                sl = slice(i * 512, (i + 1) * 512)
                nc.vector.tensor_copy(G_v[:, sl], pgv[i][:])
            eng = nc.sync if i % 2 == 0 else nc.scalar
            eng.dma_start(oh[osl, :], rh[:])
        sl = slice(7 * 512, 8 * 512)
        nc.vector.tensor_copy(G_v[:, sl], pgv[7][:])
        for i in range(NT):
            osl = slice(i * P, (i + 1) * P)
            rv = res.tile([P, S], BF16, tag="res", name=f"rv{i}")
            for j in range(2):
                sl = slice(j * 2048, (j + 1) * 2048)
                nc.vector.tensor_mul(rv[:, sl], xt[i][:, sl], G_v[:, sl])
            eng = nc.sync if i % 2 == 0 else nc.scalar
            eng.dma_start(ov[osl, :], rv[:])


def _build_device_kernel():
    import concourse.bacc as bacc
    import concourse.mybir as mybir
    import concourse.tile as tile

    BF16 = mybir.dt.bfloat16
    F32 = mybir.dt.float32
    nc = bacc.Bacc("TRN2", target_bir_lowering=False, debug=False)
    x = nc.dram_tensor("x", [C, S], BF16, kind="ExternalInput").ap()
    gp = nc.dram_tensor("gp", [4, 32], F32, kind="ExternalInput").ap()
    oh = nc.dram_tensor("out_h", [C, S], BF16, kind="ExternalOutput").ap()
    ov = nc.dram_tensor("out_v", [C, S], BF16, kind="ExternalOutput").ap()

    with tile.TileContext(nc) as tc:
        _emit(tc, [oh, ov], [x, gp])

    nc.compile()
    return nc


def _get_nc():
    if "nc" not in _CACHE:
        _CACHE["nc"] = _build_device_kernel()
    return _CACHE["nc"]


def _run(inputs, **spmd_kwargs):
    """Shard, execute on 8 cores, gather. Returns (out_h, out_v, results)."""
    import ml_dtypes

    from concourse.bass_utils import run_bass_kernel_spmd

    nc = _get_nc()
    x = np.asarray(inputs["x"], dtype=np.float32)
    assert x.shape == (B, C, HW, HW), x.shape
    xb = np.ascontiguousarray(x.reshape(B, C, S).astype(ml_dtypes.bfloat16))
    gp = _pack_gate_params(inputs)
    in_maps = [{"x": xb[b], "gp": gp} for b in range(B)]
    r = run_bass_kernel_spmd(nc, in_maps, core_ids=list(range(B)), **spmd_kwargs)
    oh = np.stack([r.results[b]["out_h"] for b in range(B)])
    ov = np.stack([r.results[b]["out_v"] for b in range(B)])
    oh = oh.astype(np.float32).reshape(B, C, HW, HW)
    ov = ov.astype(np.float32).reshape(B, C, HW, HW)
    return oh, ov, r


def kernel(**inputs):
    oh, ov, _ = _run(inputs)
    return oh, ov
